# revision 1
# baseline (speedup 1.0000x reference)
"""Sequence-parallel self-attention kernel for 8 TRN2 NeuronCores.

Reference computation (N=8192, D=256, fp32):
    q = x @ WQ; k = x @ WK; v = x @ WV
    out = softmax(q @ k.T) @ v

Host->device traffic is the wall-clock bottleneck (axon tunnel ~35 MB/s), so
each core receives ONLY its own fp16 shard plus a 1/8 slice of the packed
weights (~0.55 MB/core instead of 17.8 MB/core replicated), and the full x is
reconstructed on-device with AllGathers over NeuronLink (~14 us each):

  per core c (one fused fp16 input array xw_h [1120, 256]):
    rows 0..1023     own x rows (natural layout)
    rows 1024..1119  rows c*96..(c+1)*96 of packed [WQ; WK.T; WV]
  on device:
    AG#1: cast(xs_h)->f32r, gather -> xg  [8192, 256]   (natural x)
    AG#2: XBAR dma-transpose(xs_h)->f32r, gather -> xgT [2048, 1024]
          (8 stacked [256,1024] per-core transposed shards)
    AG#3: gather w_h -> w_all [768, 256] fp16, cast -> f32r weight tiles

Per-core algebra (identical to the proven replicated-input kernel; everything
stays transposed so softmax's k-reduction is a partition-axis ones-matmul):
    qT = WQ.T @ xT_local                      [256, 1024]
    M  = WK @ qT        (lhsT = WK.T)         [256, 1024]
    per k-chunk c (64 chunks of 128):
      scoresT = x_c @ M                       [128, 1024]   (= (q @ k.T).T chunk)
      expT    = exp(scoresT - 15)             (constant shift cancels in softmax)
      sums   += ones[128,1].T @ expT          [1, 1024]     (softmax denominator)
      UT     += x_c.T @ expT                  [256, 1024]   (= (attn_unnorm @ x).T)
    UTn  = UT * broadcast(1/sums)
    outT = WV.T @ UTn                         [256, 1024]   (int8 + f32 absmax
                                                             scale; host
                                                             dequantizes + .T)

All matmuls run as float32r (full PE rate at free-dim >= 256). fp16 input
transport + f32r compute + int8 output quantization land at ~5.7e-3 rel err
-- 3.5x under the 2e-2 gate.

Hardware quirk found while tuning: late SBUF->DRAM stores issued on the sync
DMA queue corrupt their payload in this build (every 32-bit word of some 4KB
spans gets +0x800 added then its low 12 bits cleared -- an fp32-mantissa-style
rounding). The output stores therefore go through the gpsimd DMA queue, which
is unaffected.
"""

import numpy as np

N, D, P = 8192, 256, 8
NL = N // P          # 1024 q-rows per core
KC = 128             # k-chunk size (contraction tile)
NCHUNK = N // KC     # 64
SB = 8               # k-chunks per DMA superblock
WSH = 3 * D // P     # 96 packed-weight rows per core
EXP_SHIFT = -15.0    # exp(s - 15): keeps ACT exp-table args in a good range
QDEN = 120.0         # int8 quant denominator; headroom vs 127 absorbs the
                     # ~1% error of the DVE reciprocal so +max never wraps

_CACHE = {}


def _build():
    import concourse.bacc as bacc
    import concourse.mybir as mybir
    import concourse.tile as tile

    import concourse.bass_isa as bass_isa

    f32 = mybir.dt.float32
    f32r = mybir.dt.float32r
    f16 = mybir.dt.float16
    i8 = mybir.dt.int8
    EXP = mybir.ActivationFunctionType.Exp
    COPY = mybir.ActivationFunctionType.Copy
    RG = [list(range(P))]

    nc = bacc.Bacc("TRN2", target_bir_lowering=False, debug=False,
                   enable_asserts=False, num_devices=P,
                   enable_partition_id=False)

    xw_h = nc.dram_tensor("xw_h", [NL + WSH, D], f16, kind="ExternalInput").ap()
    xs_h = xw_h[0:NL, :]
    w_hs = xw_h[NL:NL + WSH, :]
    outT = nc.dram_tensor("outT", [D + 1, NL], i8, kind="ExternalOutput").ap()

    with tile.TileContext(nc) as tc:
        with (
            tc.tile_pool(name="const", bufs=1) as cpool,
            tc.tile_pool(name="proj", bufs=1) as ppool,
            tc.tile_pool(name="xts", bufs=4) as xtpool,
            tc.tile_pool(name="xns", bufs=4) as xnpool,
            tc.tile_pool(name="expt", bufs=8) as epool,
            tc.tile_pool(name="tail", bufs=1) as tpool,
            tc.tile_pool(name="dram", bufs=1, space="DRAM") as dpool,
            tc.tile_pool(name="ps_scores", bufs=2, space="PSUM") as ps_s,
            tc.tile_pool(name="ps_ut", bufs=1, space="PSUM") as ps_ut,
            tc.tile_pool(name="ps_sums", bufs=1, space="PSUM") as ps_sum,
        ):
            # ---- stage own shard + weights into DRAM, AllGather ----
            # natural-layout shard: fp16 -> SBUF -> f32r -> local DRAM -> AG
            xs_sb_h = cpool.tile([128, SB, D], f16, tag="xs_sb_h", name="xs_sb_h")
            nc.sync.dma_start(
                xs_sb_h[:], xs_h[:].rearrange("(a p) d -> p a d", p=128))
            xs_sb = cpool.tile([128, SB, D], f32r, tag="xs_sb", name="xs_sb")
            nc.vector.tensor_copy(xs_sb[:], xs_sb_h[:])
            xs_int = dpool.tile([NL, D], f32r, tag="xs_int", name="xs_int")
            nc.sync.dma_start(
                xs_int[:].rearrange("(a p) d -> p a d", p=128), xs_sb[:])
            xg = dpool.tile([N, D], f32r, tag="xg", name="xg",
                            addr_space="Shared")
            nc.gpsimd.collective_compute(
                "AllGather", mybir.AluOpType.bypass, replica_groups=RG,
                ins=[xs_int.opt()], outs=[xg.opt()])

            # transposed shard via XBAR dma-transpose: fp16 -> f32r -> AG
            xsT_sb = []
            xsT_int = dpool.tile([D, NL], f32r, tag="xsT_int", name="xsT_int")
            for h in range(2):
                trh = cpool.tile([128, NL], f16, tag=f"trh{h}", name=f"trh{h}")
                nc.sync.dma_start(
                    trh[:], xs_h[:, h * 128:(h + 1) * 128], transpose=True)
                trf = cpool.tile([128, NL], f32r, tag=f"trf{h}", name=f"trf{h}")
                nc.vector.tensor_copy(trf[:], trh[:])
                nc.sync.dma_start(xsT_int[h * 128:(h + 1) * 128, :], trf[:])
                xsT_sb.append(trf)
            xgT = dpool.tile([P * D, NL], f32r, tag="xgT", name="xgT",
                             addr_space="Shared")
            nc.gpsimd.collective_compute(
                "AllGather", mybir.AluOpType.bypass, replica_groups=RG,
                ins=[xsT_int.opt()], outs=[xgT.opt()])

            # packed weights [WQ; WK.T; WV]: shard -> AG -> SBUF f32r tiles
            w_sb_h = cpool.tile([WSH, D], f16, tag="w_sb_h", name="w_sb_h")
            nc.sync.dma_start(w_sb_h[:], w_hs)
            w_int = dpool.tile([WSH, D], f16, tag="w_int", name="w_int")
            nc.sync.dma_start(w_int[:], w_sb_h[:])
            w_all = dpool.tile([3 * D, D], f16, tag="w_all", name="w_all",
                               addr_space="Shared")
            nc.gpsimd.collective_compute(
                "AllGather", mybir.AluOpType.bypass, replica_groups=RG,
                ins=[w_int.opt()], outs=[w_all.opt()])

            def wtiles(base, nm):
                out = []
                for h in range(2):
                    th = cpool.tile([128, D], f16, tag=f"{nm}h{h}",
                                    name=f"{nm}h{h}")
                    nc.sync.dma_start(
                        th[:], w_all[base + h * 128: base + (h + 1) * 128, :])
                    tf = cpool.tile([128, D], f32r, tag=f"{nm}{h}",
                                    name=f"{nm}{h}")
                    nc.vector.tensor_copy(tf[:], th[:])
                    out.append(tf)
                return out

            wq_t = wtiles(0, "wq")
            wkt_t = wtiles(D, "wkt")
            wv_t = wtiles(2 * D, "wv")

            # ---- constants ----
            ones_col = cpool.tile([128, 1], f32r, tag="ones_col", name="ones_col")
            ones_row = cpool.tile([1, 128], f32r, tag="ones_row", name="ones_row")
            ones_f32 = cpool.tile([128, 1], f32, tag="ones_f32", name="ones_f32")
            ones_f32r = cpool.tile([1, 128], f32, tag="ones_f32r", name="ones_f32r")
            bias_t = cpool.tile([128, 1], f32, tag="bias_t", name="bias_t")
            nc.vector.memset(ones_f32[:], 1.0)
            nc.vector.memset(ones_f32r[:], 1.0)
            nc.vector.tensor_copy(ones_col[:], ones_f32[:])
            nc.vector.tensor_copy(ones_row[:], ones_f32r[:])
            nc.vector.memset(bias_t[:], EXP_SHIFT)

            # ---- qT = WQ.T @ xT_local ; M = WK @ qT ----
            qT_t = [ppool.tile([128, NL], f32r, tag=f"qt{h}", name=f"qt{h}") for h in range(2)]
            m_t = [ppool.tile([128, NL], f32r, tag=f"m{h}", name=f"m{h}") for h in range(2)]
            for dst, lhs in ((qT_t, wq_t), (m_t, wkt_t)):
                src = xsT_sb if dst is qT_t else qT_t
                for mh in range(2):
                    for nh in range(2):
                        pp = ps_s.tile([128, 512], f32, tag="scores", name="scores")
                        for kp in range(2):
                            nc.tensor.matmul(
                                pp[:],
                                lhs[kp][:, mh * 128:(mh + 1) * 128],
                                src[kp][:, nh * 512:(nh + 1) * 512],
                                start=(kp == 0), stop=(kp == 1),
                            )
                        nc.vector.tensor_copy(
                            dst[mh][:, nh * 512:(nh + 1) * 512], pp[:])

            # ---- persistent accumulators ----
            ut_ps = [ps_ut.tile([128, NL], f32, tag=f"ut{h}", name=f"ut{h}") for h in range(2)]
            sums_ps = [ps_sum.tile([1, 512], f32, tag=f"sums{h}", name=f"sums{h}")
                       for h in range(2)]

            # ---- main k-loop ----
            for sb in range(N // (KC * SB)):
                xt_t = [xtpool.tile([128, KC * SB], f32r, tag=f"xt{h}", name=f"xt{h}")
                        for h in range(2)]
                for h in range(2):
                    nc.sync.dma_start(
                        xt_t[h][:],
                        xgT[sb * 2 * 128 + h * 128:sb * 2 * 128 + (h + 1) * 128,
                            :])
                xn_t = xnpool.tile([128, SB, D], f32r, tag="xn", name="xn")
                nc.sync.dma_start(
                    xn_t[:],
                    xg[sb * KC * SB:(sb + 1) * KC * SB, :]
                    .rearrange("(a p) d -> p a d", p=128))

                for j in range(SB):
                    c = sb * SB + j
                    first, last = (c == 0), (c == NCHUNK - 1)
                    exps = []
                    for qh in range(2):
                        sp = ps_s.tile([128, 512], f32, tag="scores", name="scores")
                        for kp in range(2):
                            nc.tensor.matmul(
                                sp[:],
                                xt_t[kp][:, j * KC:(j + 1) * KC],
                                m_t[kp][:, qh * 512:(qh + 1) * 512],
                                start=(kp == 0), stop=(kp == 1),
                            )
                        et = epool.tile([128, 512], f32r, tag="expt", name="expt")
                        nc.scalar.activation(et[:], sp[:], EXP, bias=bias_t[:])
                        exps.append(et)
                    for qh in range(2):
                        et = exps[qh]
                        nc.tensor.matmul(
                            sums_ps[qh][:], ones_col[:], et[:],
                            start=first, stop=last)
                        for dh in range(2):
                            nc.tensor.matmul(
                                ut_ps[dh][:, qh * 512:(qh + 1) * 512],
                                xn_t[:, j, dh * 128:(dh + 1) * 128],
                                et[:],
                                start=first, stop=last)

            # ---- tail: softmax normalize + WV projection ----
            sums_sb = tpool.tile([1, NL], f32, tag="sums_sb", name="sums_sb")
            for qh in range(2):
                nc.vector.tensor_copy(
                    sums_sb[:, qh * 512:(qh + 1) * 512], sums_ps[qh][:])
            recip_sb = tpool.tile([1, NL], f32r, tag="recip_sb", name="recip_sb")
            with nc.allow_low_precision(reason="f32r is 4-byte, same mantissa path"):
                nc.vector.reciprocal(recip_sb[:], sums_sb[:])

            rb_sb = tpool.tile([128, NL], f32, tag="rb_sb", name="rb_sb")
            for qh in range(2):
                rp = ps_s.tile([128, 512], f32, tag="scores", name="scores")
                nc.tensor.matmul(
                    rp[:], ones_row[:],
                    recip_sb[:, qh * 512:(qh + 1) * 512],
                    start=True, stop=True)
                nc.vector.tensor_copy(rb_sb[:, qh * 512:(qh + 1) * 512], rp[:])

            utn_sb = [tpool.tile([128, NL], f32r, tag=f"utn{h}", name=f"utn{h}")
                      for h in range(2)]
            for dh in range(2):
                nc.vector.tensor_mul(utn_sb[dh][:], ut_ps[dh][:], rb_sb[:])

            # WV projection into two live PSUM tiles, then int8-quantize with a
            # per-core absmax scale (int8 + f32 scale halves the output bytes;
            # quantization error ~m/240 is ~4e-3 of the rel-err denominator)
            o_f32 = []
            am = tpool.tile([128, 2], f32, tag="am", name="am")
            for mh in range(2):
                op = ps_ut.tile([128, NL], f32, tag=f"ut{mh}", name=f"ut{mh}")
                for nh in range(2):
                    for kp in range(2):
                        nc.tensor.matmul(
                            op[:, nh * 512:(nh + 1) * 512],
                            wv_t[kp][:, mh * 128:(mh + 1) * 128],
                            utn_sb[kp][:, nh * 512:(nh + 1) * 512],
                            start=(kp == 0), stop=(kp == 1),
                        )
                of = tpool.tile([128, NL], f32, tag=f"of{mh}", name=f"of{mh}")
                nc.vector.tensor_copy(of[:], op[:])
                nc.vector.reduce_max(
                    am[:, mh:mh + 1], of[:], axis=mybir.AxisListType.X,
                    apply_absolute_value=True)
                o_f32.append(of)
            amax = tpool.tile([128, 1], f32, tag="amax", name="amax")
            nc.vector.reduce_max(amax[:], am[:], axis=mybir.AxisListType.X)
            nc.gpsimd.partition_all_reduce(
                amax[:], amax[:], channels=128,
                reduce_op=bass_isa.ReduceOp.absmax)
            sc126 = tpool.tile([128, 1], f32, tag="sc126", name="sc126")
            nc.scalar.activation(sc126[:], amax[:], COPY, scale=1.0 / QDEN)
            rcp = tpool.tile([128, 1], f32, tag="rcp", name="rcp")
            nc.vector.reciprocal(rcp[:], sc126[:])
            o_sb = [tpool.tile([128, NL], i8, tag=f"osb{h}", name=f"osb{h}") for h in range(2)]
            for mh in range(2):
                with nc.allow_low_precision(reason="int8 output transport"):
                    nc.vector.tensor_scalar_mul(o_sb[mh][:], o_f32[mh][:],
                                                rcp[:])
                # gpsimd queue, NOT sync: late sync-queue stores corrupt the
                # payload in this build (32-bit words get an fp32-style
                # low-12-bit rounding); the gpsimd DGE ring is clean.
                nc.gpsimd.dma_start(
                    outT[mh * 128:(mh + 1) * 128, :], o_sb[mh][:])
            # absmax f32 bitcast to 4 bytes, packed into outT's extra row
            nc.gpsimd.dma_start(outT[D:D + 1, 0:4],
                                amax[0:1, 0:1].bitcast(i8))

    nc.compile()
    return nc


def _setup_jax_cache():
    """Persistent XLA compilation cache: run_bass_kernel_spmd re-jits a fresh
    closure every call, so without this each call pays ~100ms of XLA
    recompile for the identical HLO."""
    if "jaxcache" in _CACHE:
        return
    import jax

    jax.config.update("jax_compilation_cache_dir", "/tmp/jaxcache")
    jax.config.update("jax_persistent_cache_min_entry_size_bytes", 0)
    jax.config.update("jax_persistent_cache_min_compile_time_secs", 0)
    _CACHE["jaxcache"] = True


def _get_nc():
    if "nc" not in _CACHE:
        _setup_jax_cache()
        _CACHE["nc"] = _build()
    return _CACHE["nc"]


def make_in_maps(input, WQ, WK, WV):
    """Per-core input maps: own fp16 x shard + 1/8 of packed [WQ; WK.T; WV],
    fused into one array (fewer tunnel transfers)."""
    xh = np.ascontiguousarray(input, dtype=np.float32).astype(np.float16)
    wpack = np.concatenate(
        [np.asarray(WQ, dtype=np.float32),
         np.asarray(WK, dtype=np.float32).T,
         np.asarray(WV, dtype=np.float32)], axis=0).astype(np.float16)
    return [{
        "xw_h": np.concatenate(
            [xh[c * NL:(c + 1) * NL], wpack[c * WSH:(c + 1) * WSH]], axis=0),
    } for c in range(P)]


def kernel(input, WQ, WK, WV):
    from concourse import bass_utils

    nc = _get_nc()
    in_maps = make_in_maps(input, WQ, WK, WV)
    res = bass_utils.run_bass_kernel_spmd(nc, in_maps, core_ids=list(range(P)))
    out = np.empty((N, D), dtype=np.float32)
    for c in range(P):
        o = res.results[c]["outT"]
        amax = np.frombuffer(o[D, 0:4].tobytes(), np.float32)[0]
        out[c * NL:(c + 1) * NL, :] = (
            o[:D].astype(np.float32) * (float(amax) / QDEN)).T
    return out



# revision 2
# speedup vs baseline: 1.0629x; 1.0629x over previous
"""Sequence-parallel self-attention kernel for 8 TRN2 NeuronCores — v2.

Reference computation (N=8192, D=256, fp32):
    q = x @ WQ; k = x @ WK; v = x @ WV
    out = softmax(q @ k.T) @ v

Wall-clock is dominated by the axon tunnel (~44 MB/s, ~50 ms fixed per
round trip), so v2 attacks transport bytes on top of v1's design:

  1. x ships as 12-bit fixed point (per-row absmax scales) instead of
     fp16: int8 high bits [1024,256] + packed low nibbles [1024,128]
     + f32 row scales. 3.57 MB total vs 4.59 MB fp16. End-to-end
     numerics (CPU sim): 8.2e-3 vs the 2e-2 gate.
  2. The runner calls the bass_exec custom-call primitive directly with
     a cached jit (mirroring bass_utils.run_bass_kernel_spmd's axon
     path) and passes a persistent device-resident dummy for the outT
     operand: the NEFF never reads it (outT is renamed to output0, the
     operand slot is unbound), it only existed to pre-zero the result
     buffer via donation — which this kernel doesn't need since it
     writes every byte the host reads. Saves a 2.1 MB zeros upload and
     per-call retracing.

Per-core input blob xb [1744, 256] uint8:
    rows    0:1024  hi  = (v12 >> 4) int8   [1024,256]
    rows 1024:1536  lopack uint8 [1024,128] reshaped; lopack[:,c] =
                    lo[:,c] | lo[:,c+128]<<4  (contiguous-half packing
                    so the device unpack needs no strided writes)
    rows 1536:1552  row scales f32 laid out [128 part, 8] (s[a*128+p])
    rows 1552:1744  fp16 1/8 slice of packed [WQ; WK.T; WV]
where v12 = rint(x_row / s), s = absmax(row)/2047, x ≈ (hi*16+lo)*s.

On-device: dequant to f32r (natural layout) + fp16 scratch for the
XBAR dma-transpose path, then exactly v1's algebra: two AllGathers of
x (natural f32r + transposed f32r), one of the packed weights, then
    qT = WQ.T @ xT;  M = WK @ qT
    per 128-k-chunk: scoresT = x_c @ M; expT = exp(scoresT - 15);
    sums += 1.T @ expT; UT += x_c.T @ expT
    outT = WV.T @ (UT / sums)   -> int8 + f32 absmax scale (QDEN=120)

Output stores go through the gpsimd DMA queue: late sync-queue
SBUF->DRAM stores corrupt their payload in this build (fp32-style
low-12-bit rounding on some 4KB spans).
"""

import numpy as np

N, D, P = 8192, 256, 8
NL = N // P          # 1024 q-rows per core
KC = 128             # k-chunk size (contraction tile)
NCHUNK = N // KC     # 64
SB = 8               # k-chunks per DMA superblock
WSH = 3 * D // P     # 96 packed-weight rows per core
EXP_SHIFT = -15.0    # exp(s - 15): keeps ACT exp-table args in a good range
QDEN = 120.0         # int8 quant denominator; headroom vs 127 absorbs the
                     # ~1% error of the DVE reciprocal so +max never wraps

# blob layout (rows of 256 bytes)
R_HI0, R_HI1 = 0, 1024
R_LO0, R_LO1 = 1024, 1536
R_SC0, R_SC1 = 1536, 1552
R_W0, R_W1 = 1552, 1744
RB = R_W1            # 1744 rows -> 446464 B/core

_CACHE = {}


def _build():
    import concourse.bacc as bacc
    import concourse.mybir as mybir
    import concourse.tile as tile

    import concourse.bass_isa as bass_isa

    f32 = mybir.dt.float32
    f32r = mybir.dt.float32r
    f16 = mybir.dt.float16
    i8 = mybir.dt.int8
    u8 = mybir.dt.uint8
    EXP = mybir.ActivationFunctionType.Exp
    COPY = mybir.ActivationFunctionType.Copy
    ALU = mybir.AluOpType
    RG = [list(range(P))]

    nc = bacc.Bacc("TRN2", target_bir_lowering=False, debug=False,
                   enable_asserts=False, num_devices=P,
                   enable_partition_id=False)

    xb = nc.dram_tensor("xb", [RB, D], u8, kind="ExternalInput").ap()
    outT = nc.dram_tensor("outT", [D + 1, NL], i8, kind="ExternalOutput").ap()

    hi_ap = xb[R_HI0:R_HI1, :].bitcast(i8).rearrange("(a p) c -> p a c", p=128)
    lo_ap = xb[R_LO0:R_LO1, :].rearrange(
        "(a ph) (two c) -> (ph two) a c", ph=64, two=2)
    sc_ap = xb[R_SC0:R_SC1, :].bitcast(f32).rearrange(
        "ph (pl a) -> (ph pl) a", pl=8, a=8)
    w_ap = xb[R_W0:R_W1, :].bitcast(f16).rearrange(
        "(r two) k -> r (two k)", two=2)

    with tile.TileContext(nc) as tc:
        with (
            tc.tile_pool(name="const", bufs=1) as cpool,
            tc.tile_pool(name="proj", bufs=1) as ppool,
            tc.tile_pool(name="xts", bufs=4) as xtpool,
            tc.tile_pool(name="xns", bufs=4) as xnpool,
            tc.tile_pool(name="expt", bufs=8) as epool,
            tc.tile_pool(name="tail", bufs=1) as tpool,
            tc.tile_pool(name="dram", bufs=1, space="DRAM") as dpool,
            tc.tile_pool(name="ps_scores", bufs=2, space="PSUM") as ps_s,
            tc.tile_pool(name="ps_ut", bufs=1, space="PSUM") as ps_ut,
            tc.tile_pool(name="ps_sums", bufs=1, space="PSUM") as ps_sum,
        ):
            # ---- unpack 12-bit x -> f32r natural + fp16 scratch ----
            hi_sb = cpool.tile([128, SB, D], i8, tag="hi_sb", name="hi_sb")
            lp_sb = cpool.tile([128, SB, 128], u8, tag="lp_sb", name="lp_sb")
            sc_sb = cpool.tile([128, SB], f32, tag="sc_sb", name="sc_sb")
            nc.sync.dma_start(hi_sb[:], hi_ap)
            nc.sync.dma_start(lp_sb[:], lo_ap)
            nc.sync.dma_start(sc_sb[:], sc_ap)

            hi_f = cpool.tile([128, SB, D], f32, tag="hi_f", name="hi_f")
            nc.vector.tensor_copy(hi_f[:], hi_sb[:])
            lo_e8 = cpool.tile([128, SB, 128], u8, tag="lo_e8", name="lo_e8")
            lo_o8 = cpool.tile([128, SB, 128], u8, tag="lo_o8", name="lo_o8")
            nc.vector.tensor_scalar(lo_e8[:], lp_sb[:], 15, None,
                                    ALU.bitwise_and)
            nc.vector.tensor_scalar(lo_o8[:], lp_sb[:], 4, None,
                                    ALU.logical_shift_right)
            lo_e = cpool.tile([128, SB, 128], f32, tag="lo_e", name="lo_e")
            lo_o = cpool.tile([128, SB, 128], f32, tag="lo_o", name="lo_o")
            nc.vector.tensor_copy(lo_e[:], lo_e8[:])
            nc.vector.tensor_copy(lo_o[:], lo_o8[:])
            xsum = cpool.tile([128, SB, D], f32, tag="xsum", name="xsum")
            nc.vector.scalar_tensor_tensor(
                xsum[:, :, 0:128], hi_f[:, :, 0:128], 16.0, lo_e[:],
                ALU.mult, ALU.add)
            nc.vector.scalar_tensor_tensor(
                xsum[:, :, 128:256], hi_f[:, :, 128:256], 16.0, lo_o[:],
                ALU.mult, ALU.add)
            xs_sb = cpool.tile([128, SB, D], f32r, tag="xs_sb", name="xs_sb")
            for a in range(SB):
                nc.vector.tensor_scalar_mul(
                    xs_sb[:, a, :], xsum[:, a, :], sc_sb[:, a:a + 1])
            xs16 = cpool.tile([128, SB, D], f16, tag="xs16", name="xs16")
            nc.vector.tensor_copy(xs16[:], xs_sb[:])
            xs16_dram = dpool.tile([NL, D], f16, tag="xs16_dram",
                                   name="xs16_dram")
            nc.sync.dma_start(
                xs16_dram[:].rearrange("(a p) d -> p a d", p=128), xs16[:])

            # ---- stage shard into DRAM, AllGather (natural f32r) ----
            xs_int = dpool.tile([NL, D], f32r, tag="xs_int", name="xs_int")
            nc.sync.dma_start(
                xs_int[:].rearrange("(a p) d -> p a d", p=128), xs_sb[:])
            xg = dpool.tile([N, D], f32r, tag="xg", name="xg",
                            addr_space="Shared")
            nc.gpsimd.collective_compute(
                "AllGather", mybir.AluOpType.bypass, replica_groups=RG,
                ins=[xs_int.opt()], outs=[xg.opt()])

            # transposed shard via XBAR dma-transpose: fp16 -> f32r -> AG
            xsT_sb = []
            xsT_int = dpool.tile([D, NL], f32r, tag="xsT_int", name="xsT_int")
            for h in range(2):
                trh = cpool.tile([128, NL], f16, tag=f"trh{h}", name=f"trh{h}")
                nc.sync.dma_start(
                    trh[:], xs16_dram[:, h * 128:(h + 1) * 128],
                    transpose=True)
                trf = cpool.tile([128, NL], f32r, tag=f"trf{h}", name=f"trf{h}")
                nc.vector.tensor_copy(trf[:], trh[:])
                nc.sync.dma_start(xsT_int[h * 128:(h + 1) * 128, :], trf[:])
                xsT_sb.append(trf)
            xgT = dpool.tile([P * D, NL], f32r, tag="xgT", name="xgT",
                             addr_space="Shared")
            nc.gpsimd.collective_compute(
                "AllGather", mybir.AluOpType.bypass, replica_groups=RG,
                ins=[xsT_int.opt()], outs=[xgT.opt()])

            # packed weights [WQ; WK.T; WV]: shard -> AG -> SBUF f32r tiles
            w_sb_h = cpool.tile([WSH, D], f16, tag="w_sb_h", name="w_sb_h")
            nc.sync.dma_start(w_sb_h[:], w_ap)
            w_int = dpool.tile([WSH, D], f16, tag="w_int", name="w_int")
            nc.sync.dma_start(w_int[:], w_sb_h[:])
            w_all = dpool.tile([3 * D, D], f16, tag="w_all", name="w_all",
                               addr_space="Shared")
            nc.gpsimd.collective_compute(
                "AllGather", mybir.AluOpType.bypass, replica_groups=RG,
                ins=[w_int.opt()], outs=[w_all.opt()])

            def wtiles(base, nm):
                out = []
                for h in range(2):
                    th = cpool.tile([128, D], f16, tag=f"{nm}h{h}",
                                    name=f"{nm}h{h}")
                    nc.sync.dma_start(
                        th[:], w_all[base + h * 128: base + (h + 1) * 128, :])
                    tf = cpool.tile([128, D], f32r, tag=f"{nm}{h}",
                                    name=f"{nm}{h}")
                    nc.vector.tensor_copy(tf[:], th[:])
                    out.append(tf)
                return out

            wq_t = wtiles(0, "wq")
            wkt_t = wtiles(D, "wkt")
            wv_t = wtiles(2 * D, "wv")

            # ---- constants ----
            ones_col = cpool.tile([128, 1], f32r, tag="ones_col", name="ones_col")
            ones_row = cpool.tile([1, 128], f32r, tag="ones_row", name="ones_row")
            ones_f32 = cpool.tile([128, 1], f32, tag="ones_f32", name="ones_f32")
            ones_f32r = cpool.tile([1, 128], f32, tag="ones_f32r", name="ones_f32r")
            bias_t = cpool.tile([128, 1], f32, tag="bias_t", name="bias_t")
            nc.vector.memset(ones_f32[:], 1.0)
            nc.vector.memset(ones_f32r[:], 1.0)
            nc.vector.tensor_copy(ones_col[:], ones_f32[:])
            nc.vector.tensor_copy(ones_row[:], ones_f32r[:])
            nc.vector.memset(bias_t[:], EXP_SHIFT)

            # ---- qT = WQ.T @ xT_local ; M = WK @ qT ----
            qT_t = [ppool.tile([128, NL], f32r, tag=f"qt{h}", name=f"qt{h}") for h in range(2)]
            m_t = [ppool.tile([128, NL], f32r, tag=f"m{h}", name=f"m{h}") for h in range(2)]
            for dst, lhs in ((qT_t, wq_t), (m_t, wkt_t)):
                src = xsT_sb if dst is qT_t else qT_t
                for mh in range(2):
                    for nh in range(2):
                        pp = ps_s.tile([128, 512], f32, tag="scores", name="scores")
                        for kp in range(2):
                            nc.tensor.matmul(
                                pp[:],
                                lhs[kp][:, mh * 128:(mh + 1) * 128],
                                src[kp][:, nh * 512:(nh + 1) * 512],
                                start=(kp == 0), stop=(kp == 1),
                            )
                        nc.vector.tensor_copy(
                            dst[mh][:, nh * 512:(nh + 1) * 512], pp[:])

            # ---- persistent accumulators ----
            ut_ps = [ps_ut.tile([128, NL], f32, tag=f"ut{h}", name=f"ut{h}") for h in range(2)]
            sums_ps = [ps_sum.tile([1, 512], f32, tag=f"sums{h}", name=f"sums{h}")
                       for h in range(2)]

            # ---- main k-loop ----
            for sb in range(N // (KC * SB)):
                xt_t = [xtpool.tile([128, KC * SB], f32r, tag=f"xt{h}", name=f"xt{h}")
                        for h in range(2)]
                for h in range(2):
                    nc.sync.dma_start(
                        xt_t[h][:],
                        xgT[sb * 2 * 128 + h * 128:sb * 2 * 128 + (h + 1) * 128,
                            :])
                xn_t = xnpool.tile([128, SB, D], f32r, tag="xn", name="xn")
                nc.sync.dma_start(
                    xn_t[:],
                    xg[sb * KC * SB:(sb + 1) * KC * SB, :]
                    .rearrange("(a p) d -> p a d", p=128))

                for j in range(SB):
                    c = sb * SB + j
                    first, last = (c == 0), (c == NCHUNK - 1)
                    exps = []
                    for qh in range(2):
                        sp = ps_s.tile([128, 512], f32, tag="scores", name="scores")
                        for kp in range(2):
                            nc.tensor.matmul(
                                sp[:],
                                xt_t[kp][:, j * KC:(j + 1) * KC],
                                m_t[kp][:, qh * 512:(qh + 1) * 512],
                                start=(kp == 0), stop=(kp == 1),
                            )
                        et = epool.tile([128, 512], f32r, tag="expt", name="expt")
                        nc.scalar.activation(et[:], sp[:], EXP, bias=bias_t[:])
                        exps.append(et)
                    for qh in range(2):
                        et = exps[qh]
                        nc.tensor.matmul(
                            sums_ps[qh][:], ones_col[:], et[:],
                            start=first, stop=last)
                        for dh in range(2):
                            nc.tensor.matmul(
                                ut_ps[dh][:, qh * 512:(qh + 1) * 512],
                                xn_t[:, j, dh * 128:(dh + 1) * 128],
                                et[:],
                                start=first, stop=last)

            # ---- tail: softmax normalize + WV projection ----
            sums_sb = tpool.tile([1, NL], f32, tag="sums_sb", name="sums_sb")
            for qh in range(2):
                nc.vector.tensor_copy(
                    sums_sb[:, qh * 512:(qh + 1) * 512], sums_ps[qh][:])
            recip_sb = tpool.tile([1, NL], f32r, tag="recip_sb", name="recip_sb")
            with nc.allow_low_precision(reason="f32r is 4-byte, same mantissa path"):
                nc.vector.reciprocal(recip_sb[:], sums_sb[:])

            rb_sb = tpool.tile([128, NL], f32, tag="rb_sb", name="rb_sb")
            for qh in range(2):
                rp = ps_s.tile([128, 512], f32, tag="scores", name="scores")
                nc.tensor.matmul(
                    rp[:], ones_row[:],
                    recip_sb[:, qh * 512:(qh + 1) * 512],
                    start=True, stop=True)
                nc.vector.tensor_copy(rb_sb[:, qh * 512:(qh + 1) * 512], rp[:])

            utn_sb = [tpool.tile([128, NL], f32r, tag=f"utn{h}", name=f"utn{h}")
                      for h in range(2)]
            for dh in range(2):
                nc.vector.tensor_mul(utn_sb[dh][:], ut_ps[dh][:], rb_sb[:])

            # WV projection into two live PSUM tiles, then int8-quantize with a
            # per-core absmax scale (int8 + f32 scale halves the output bytes;
            # quantization error ~m/240 is ~4e-3 of the rel-err denominator)
            o_f32 = []
            am = tpool.tile([128, 2], f32, tag="am", name="am")
            for mh in range(2):
                op = ps_ut.tile([128, NL], f32, tag=f"ut{mh}", name=f"ut{mh}")
                for nh in range(2):
                    for kp in range(2):
                        nc.tensor.matmul(
                            op[:, nh * 512:(nh + 1) * 512],
                            wv_t[kp][:, mh * 128:(mh + 1) * 128],
                            utn_sb[kp][:, nh * 512:(nh + 1) * 512],
                            start=(kp == 0), stop=(kp == 1),
                        )
                of = tpool.tile([128, NL], f32, tag=f"of{mh}", name=f"of{mh}")
                nc.vector.tensor_copy(of[:], op[:])
                nc.vector.reduce_max(
                    am[:, mh:mh + 1], of[:], axis=mybir.AxisListType.X,
                    apply_absolute_value=True)
                o_f32.append(of)
            amax = tpool.tile([128, 1], f32, tag="amax", name="amax")
            nc.vector.reduce_max(amax[:], am[:], axis=mybir.AxisListType.X)
            nc.gpsimd.partition_all_reduce(
                amax[:], amax[:], channels=128,
                reduce_op=bass_isa.ReduceOp.absmax)
            sc126 = tpool.tile([128, 1], f32, tag="sc126", name="sc126")
            nc.scalar.activation(sc126[:], amax[:], COPY, scale=1.0 / QDEN)
            rcp = tpool.tile([128, 1], f32, tag="rcp", name="rcp")
            nc.vector.reciprocal(rcp[:], sc126[:])
            o_sb = [tpool.tile([128, NL], i8, tag=f"osb{h}", name=f"osb{h}") for h in range(2)]
            for mh in range(2):
                with nc.allow_low_precision(reason="int8 output transport"):
                    nc.vector.tensor_scalar_mul(o_sb[mh][:], o_f32[mh][:],
                                                rcp[:])
                # gpsimd queue, NOT sync: late sync-queue stores corrupt the
                # payload in this build (32-bit words get an fp32-style
                # low-12-bit rounding); the gpsimd DGE ring is clean.
                nc.gpsimd.dma_start(
                    outT[mh * 128:(mh + 1) * 128, :], o_sb[mh][:])
            # absmax f32 bitcast to 4 bytes, packed into outT's extra row
            nc.gpsimd.dma_start(outT[D:D + 1, 0:4],
                                amax[0:1, 0:1].bitcast(i8))

    nc.compile()
    return nc


def _setup_jax_cache():
    """Persistent XLA compilation cache so a fresh process reuses the
    compiled NEFF-wrapped executable instead of re-lowering."""
    if "jaxcache" in _CACHE:
        return
    import jax

    jax.config.update("jax_compilation_cache_dir", "/tmp/jaxcache")
    jax.config.update("jax_persistent_cache_min_entry_size_bytes", 0)
    jax.config.update("jax_persistent_cache_min_compile_time_secs", 0)
    _CACHE["jaxcache"] = True


def _get_nc():
    if "nc" not in _CACHE:
        _setup_jax_cache()
        _CACHE["nc"] = _build()
    return _CACHE["nc"]


def _get_runner():
    """Cached jit of the bass_exec custom call over 8 cores + persistent
    device-resident dummy for the outT operand (never read by the NEFF;
    see module docstring). Mirrors run_bass_kernel_spmd's axon path."""
    if "runner" in _CACHE:
        return _CACHE["runner"]
    import jax
    import concourse.mybir as mybir
    from concourse.bass2jax import _bass_exec_p, install_neuronx_cc_hook
    from jax.experimental.shard_map import shard_map
    from jax.sharding import Mesh, NamedSharding, PartitionSpec

    nc = _get_nc()
    install_neuronx_cc_hook()

    in_names, out_names, out_avals = [], [], []
    for alloc in nc.m.functions[0].allocations:
        if not isinstance(alloc, mybir.MemoryLocationSet):
            continue
        name = alloc.memorylocations[0].name
        if alloc.kind == "ExternalInput":
            in_names.append(name)
        elif alloc.kind == "ExternalOutput":
            out_names.append(name)
            out_avals.append(jax.core.ShapedArray(
                tuple(alloc.tensor_shape), mybir.dt.np(alloc.dtype)))
    in_names = in_names + out_names

    def _body(*args):
        return tuple(_bass_exec_p.bind(
            *args,
            out_avals=tuple(out_avals),
            in_names=tuple(in_names),
            out_names=tuple(out_names),
            lowering_input_output_aliases=(),
            sim_require_finite=True,
            sim_require_nnan=True,
            nc=nc,
        ))

    devices = jax.devices()[:P]
    mesh = Mesh(np.asarray(devices), ("core",))
    fn = jax.jit(shard_map(_body, mesh=mesh,
                           in_specs=(PartitionSpec("core"),) * len(in_names),
                           out_specs=(PartitionSpec("core"),) * len(out_names),
                           check_rep=False),
                 keep_unused=True)
    oav = out_avals[0]
    dummy = jax.device_put(
        np.zeros((P * oav.shape[0], *oav.shape[1:]), oav.dtype),
        NamedSharding(mesh, PartitionSpec("core")))
    dummy.block_until_ready()
    _CACHE["runner"] = (fn, dummy)
    return _CACHE["runner"]


def make_blob(input, WQ, WK, WV):
    """Global [8*1744, 256] uint8 input blob (all cores concatenated)."""
    x = np.ascontiguousarray(input, dtype=np.float32)
    s = np.abs(x).max(axis=1)
    s /= 2047.0
    v12 = np.rint(x * (1.0 / s)[:, None]).astype(np.int16)
    hi = (v12 >> 4).astype(np.int8)
    lo = (v12 & 15).astype(np.uint8)
    lopack = lo[:, :128] | (lo[:, 128:] << 4)
    wpack = np.concatenate(
        [np.asarray(WQ, dtype=np.float32),
         np.asarray(WK, dtype=np.float32).T,
         np.asarray(WV, dtype=np.float32)], axis=0).astype(np.float16)

    blob = np.empty((P * RB, D), np.uint8)
    for c in range(P):
        b = blob[c * RB:(c + 1) * RB]
        r0, r1 = c * NL, (c + 1) * NL
        b[R_HI0:R_HI1] = hi[r0:r1].view(np.uint8)
        b[R_LO0:R_LO1] = lopack[r0:r1].reshape(512, 256)
        b[R_SC0:R_SC1] = np.ascontiguousarray(
            s[r0:r1].reshape(SB, 128).T).view(np.uint8).reshape(16, 256)
        b[R_W0:R_W1] = wpack[c * WSH:(c + 1) * WSH].view(np.uint8).reshape(192, 256)
    return blob


def _execute(blob):
    """The timed hot path: one jit call (h2d blob, NEFF exec, d2h outT)."""
    fn, dummy = _get_runner()
    out = fn(blob, dummy)
    return np.asarray(out[0])


def _unshard(o_all):
    res = o_all.reshape(P, D + 1, NL)
    out = np.empty((N, D), dtype=np.float32)
    for c in range(P):
        o = res[c]
        amax = np.frombuffer(o[D, 0:4].tobytes(), np.float32)[0]
        out[c * NL:(c + 1) * NL, :] = (
            o[:D].astype(np.float32) * (float(amax) / QDEN)).T
    return out


def kernel(input, WQ, WK, WV):
    _get_runner()
    blob = make_blob(input, WQ, WK, WV)
    return _unshard(_execute(blob))


# revision 8
# speedup vs baseline: 1.0920x; 1.0274x over previous
"""Sequence-parallel self-attention kernel for 8 TRN2 NeuronCores — v2.

Reference computation (N=8192, D=256, fp32):
    q = x @ WQ; k = x @ WK; v = x @ WV
    out = softmax(q @ k.T) @ v

Wall-clock is dominated by the axon tunnel (~44 MB/s, ~50 ms fixed per
round trip), so v2 attacks transport bytes on top of v1's design:

  1. x ships as 12-bit fixed point (per-row absmax scales) instead of
     fp16: int8 high bits [1024,256] + packed low nibbles [1024,128]
     + f32 row scales. 3.57 MB total vs 4.59 MB fp16. End-to-end
     numerics (CPU sim): 8.2e-3 vs the 2e-2 gate.
  2. The runner calls the bass_exec custom-call primitive directly with
     a cached jit (mirroring bass_utils.run_bass_kernel_spmd's axon
     path) and passes a persistent device-resident dummy for the outT
     operand: the NEFF never reads it (outT is renamed to output0, the
     operand slot is unbound), it only existed to pre-zero the result
     buffer via donation — which this kernel doesn't need since it
     writes every byte the host reads. Saves a 2.1 MB zeros upload and
     per-call retracing.

Per-core input blob xb [1698, 256] uint8:
    rows    0:1024  x hi = (v12 >> 4) int8  [1024,256]
    rows 1024:1536  x lopack uint8 [1024,128] reshaped; lopack[:,c] =
                    lo[:,c] | lo[:,c+128]<<4  (contiguous-half packing
                    so the device unpack needs no strided writes)
    rows 1536:1552  x row scales f32 laid out [128 part, 8] (s[a*128+p])
    rows 1552:1698  1/8 slice of packed [WQ; WK.T; WV], same 12-bit
                    per-row format (hi [96,256] / lopack [96,128] /
                    row scales f32 [96] padded to 128)
where v12 = rint(row / s), s = absmax(row)/2047, row ≈ (hi*16+lo)*s.
12-bit weights cost nothing numerically (CPU sim: 8.185e-3 vs 8.190e-3
with fp16 weights).

On-device: dequant to f32r (natural layout) + fp16 scratch for the
XBAR dma-transpose path, then exactly v1's algebra: two AllGathers of
x (natural f32r + transposed f32r), one of the packed weights, then
    qT = WQ.T @ xT;  M = WK @ qT
    per 128-k-chunk: scoresT = x_c @ M; expT = exp(scoresT - 15);
    sums += 1.T @ expT; UT += x_c.T @ expT
    outT = WV.T @ (UT / sums)   -> int8 + f32 absmax scale (QDEN=120)

Output stores go through the gpsimd DMA queue: late sync-queue
SBUF->DRAM stores corrupt their payload in this build (fp32-style
low-12-bit rounding on some 4KB spans).
"""

import numpy as np

N, D, P = 8192, 256, 8
NL = N // P          # 1024 q-rows per core
KC = 128             # k-chunk size (contraction tile)
NCHUNK = N // KC     # 64
SB = 8               # k-chunks per DMA superblock
WSH = 3 * D // P     # 96 packed-weight rows per core
EXP_SHIFT = -15.0    # exp(s - 15): keeps ACT exp-table args in a good range
QDEN = 120.0         # int8 quant denominator; headroom vs 127 absorbs the
                     # ~1% error of the DVE reciprocal so +max never wraps

# blob layout (rows of 256 bytes)
R_HI0, R_HI1 = 0, 1024       # x high 8 bits, int8 [1024,256]
R_LO0, R_LO1 = 1024, 1536    # x low nibbles packed [1024,128]
R_SC0, R_SC1 = 1536, 1552    # x row scales f32 [128,8]
R_WH0, R_WH1 = 1552, 1648    # w high 8 bits, int8 [96,256]
R_WL0, R_WL1 = 1648, 1696    # w low nibbles packed [96,128]
R_WS0, R_WS1 = 1696, 1698    # w row scales f32 [96] (padded to 128)
RB = R_WS1           # 1698 rows -> 434688 B/core

_CACHE = {}


def _build():
    import concourse.bacc as bacc
    import concourse.mybir as mybir
    import concourse.tile as tile

    import concourse.bass_isa as bass_isa

    f32 = mybir.dt.float32
    f32r = mybir.dt.float32r
    f16 = mybir.dt.float16
    i8 = mybir.dt.int8
    u8 = mybir.dt.uint8
    EXP = mybir.ActivationFunctionType.Exp
    COPY = mybir.ActivationFunctionType.Copy
    ALU = mybir.AluOpType
    RG = [list(range(P))]

    nc = bacc.Bacc("TRN2", target_bir_lowering=False, debug=False,
                   enable_asserts=False, num_devices=P,
                   enable_partition_id=False)

    xb = nc.dram_tensor("xb", [RB, D], u8, kind="ExternalInput").ap()
    outT = nc.dram_tensor("outT", [D + 1, NL], i8, kind="ExternalOutput").ap()

    hi_ap = xb[R_HI0:R_HI1, :].bitcast(i8).rearrange("(a p) c -> p a c", p=128)
    lo_ap = xb[R_LO0:R_LO1, :].rearrange(
        "(a ph) (two c) -> (ph two) a c", ph=64, two=2)
    sc_ap = xb[R_SC0:R_SC1, :].bitcast(f32).rearrange(
        "ph (pl a) -> (ph pl) a", pl=8, a=8)
    whi_ap = xb[R_WH0:R_WH1, :].bitcast(i8)
    wlo_ap = xb[R_WL0:R_WL1, :].rearrange(
        "ph (two c) -> (ph two) c", two=2)
    wsc_ap = xb[R_WS0:R_WS1, :].bitcast(f32).rearrange(
        "ph (pl a) -> (ph pl) a", pl=64, a=1)

    with tile.TileContext(nc) as tc:
        with (
            tc.tile_pool(name="const", bufs=1) as cpool,
            tc.tile_pool(name="proj", bufs=1) as ppool,
            tc.tile_pool(name="xts", bufs=4) as xtpool,
            tc.tile_pool(name="xns", bufs=4) as xnpool,
            tc.tile_pool(name="expt", bufs=8) as epool,
            tc.tile_pool(name="tail", bufs=1) as tpool,
            tc.tile_pool(name="dram", bufs=1, space="DRAM") as dpool,
            tc.tile_pool(name="ps_scores", bufs=2, space="PSUM") as ps_s,
            tc.tile_pool(name="ps_ut", bufs=1, space="PSUM") as ps_ut,
            tc.tile_pool(name="ps_sums", bufs=1, space="PSUM") as ps_sum,
        ):
            # ---- unpack 12-bit x -> f32r natural + fp16 scratch ----
            hi_sb = cpool.tile([128, SB, D], i8, tag="hi_sb", name="hi_sb")
            lp_sb = cpool.tile([128, SB, 128], u8, tag="lp_sb", name="lp_sb")
            sc_sb = cpool.tile([128, SB], f32, tag="sc_sb", name="sc_sb")
            nc.sync.dma_start(hi_sb[:], hi_ap)
            nc.sync.dma_start(lp_sb[:], lo_ap)
            nc.sync.dma_start(sc_sb[:], sc_ap)

            hi_f = cpool.tile([128, SB, D], f32, tag="hi_f", name="hi_f")
            nc.vector.tensor_copy(hi_f[:], hi_sb[:])
            lo_e8 = cpool.tile([128, SB, 128], u8, tag="lo_e8", name="lo_e8")
            lo_o8 = cpool.tile([128, SB, 128], u8, tag="lo_o8", name="lo_o8")
            nc.vector.tensor_scalar(lo_e8[:], lp_sb[:], 15, None,
                                    ALU.bitwise_and)
            nc.vector.tensor_scalar(lo_o8[:], lp_sb[:], 4, None,
                                    ALU.logical_shift_right)
            lo_e = cpool.tile([128, SB, 128], f32, tag="lo_e", name="lo_e")
            lo_o = cpool.tile([128, SB, 128], f32, tag="lo_o", name="lo_o")
            nc.vector.tensor_copy(lo_e[:], lo_e8[:])
            nc.vector.tensor_copy(lo_o[:], lo_o8[:])
            xsum = cpool.tile([128, SB, D], f32, tag="xsum", name="xsum")
            nc.vector.scalar_tensor_tensor(
                xsum[:, :, 0:128], hi_f[:, :, 0:128], 16.0, lo_e[:],
                ALU.mult, ALU.add)
            nc.vector.scalar_tensor_tensor(
                xsum[:, :, 128:256], hi_f[:, :, 128:256], 16.0, lo_o[:],
                ALU.mult, ALU.add)
            xs_sb = cpool.tile([128, SB, D], f32r, tag="xs_sb", name="xs_sb")
            for a in range(SB):
                nc.vector.tensor_scalar_mul(
                    xs_sb[:, a, :], xsum[:, a, :], sc_sb[:, a:a + 1])
            xs16 = cpool.tile([128, SB, D], f16, tag="xs16", name="xs16")
            nc.vector.tensor_copy(xs16[:], xs_sb[:])
            xs16_dram = dpool.tile([NL, D], f16, tag="xs16_dram",
                                   name="xs16_dram")
            nc.sync.dma_start(
                xs16_dram[:].rearrange("(a p) d -> p a d", p=128), xs16[:])

            # ---- stage shard into DRAM, AllGather (natural f32r) ----
            xs_int = dpool.tile([NL, D], f32r, tag="xs_int", name="xs_int")
            nc.sync.dma_start(
                xs_int[:].rearrange("(a p) d -> p a d", p=128), xs_sb[:])
            xg = dpool.tile([N, D], f32r, tag="xg", name="xg",
                            addr_space="Shared")
            nc.gpsimd.collective_compute(
                "AllGather", mybir.AluOpType.bypass, replica_groups=RG,
                ins=[xs_int.opt()], outs=[xg.opt()])

            # transposed shard via XBAR dma-transpose: fp16 -> f32r -> AG
            xsT_sb = []
            xsT_int = dpool.tile([D, NL], f32r, tag="xsT_int", name="xsT_int")
            for h in range(2):
                trh = cpool.tile([128, NL], f16, tag=f"trh{h}", name=f"trh{h}")
                nc.sync.dma_start(
                    trh[:], xs16_dram[:, h * 128:(h + 1) * 128],
                    transpose=True)
                trf = cpool.tile([128, NL], f32r, tag=f"trf{h}", name=f"trf{h}")
                nc.vector.tensor_copy(trf[:], trh[:])
                nc.sync.dma_start(xsT_int[h * 128:(h + 1) * 128, :], trf[:])
                xsT_sb.append(trf)
            xgT = dpool.tile([P * D, NL], f32r, tag="xgT", name="xgT",
                             addr_space="Shared")
            nc.gpsimd.collective_compute(
                "AllGather", mybir.AluOpType.bypass, replica_groups=RG,
                ins=[xsT_int.opt()], outs=[xgT.opt()])

            # packed weights [WQ; WK.T; WV]: 12-bit shard -> unpack to f16
            # -> AG -> SBUF f32r tiles
            whi_sb = cpool.tile([WSH, D], i8, tag="whi_sb", name="whi_sb")
            wlo_sb = cpool.tile([WSH, 128], u8, tag="wlo_sb", name="wlo_sb")
            wsc_sb = cpool.tile([128, 1], f32, tag="wsc_sb", name="wsc_sb")
            nc.sync.dma_start(whi_sb[:], whi_ap)
            nc.sync.dma_start(wlo_sb[:], wlo_ap)
            nc.sync.dma_start(wsc_sb[:], wsc_ap)
            whi_f = cpool.tile([WSH, D], f32, tag="whi_f", name="whi_f")
            nc.vector.tensor_copy(whi_f[:], whi_sb[:])
            we8 = cpool.tile([WSH, 128], u8, tag="we8", name="we8")
            wo8 = cpool.tile([WSH, 128], u8, tag="wo8", name="wo8")
            nc.vector.tensor_scalar(we8[:], wlo_sb[:], 15, None,
                                    ALU.bitwise_and)
            nc.vector.tensor_scalar(wo8[:], wlo_sb[:], 4, None,
                                    ALU.logical_shift_right)
            we_f = cpool.tile([WSH, 128], f32, tag="we_f", name="we_f")
            wo_f = cpool.tile([WSH, 128], f32, tag="wo_f", name="wo_f")
            nc.vector.tensor_copy(we_f[:], we8[:])
            nc.vector.tensor_copy(wo_f[:], wo8[:])
            wsum = cpool.tile([WSH, D], f32, tag="wsum", name="wsum")
            nc.vector.scalar_tensor_tensor(
                wsum[:, 0:128], whi_f[:, 0:128], 16.0, we_f[:],
                ALU.mult, ALU.add)
            nc.vector.scalar_tensor_tensor(
                wsum[:, 128:256], whi_f[:, 128:256], 16.0, wo_f[:],
                ALU.mult, ALU.add)
            w_sb_h = cpool.tile([WSH, D], f16, tag="w_sb_h", name="w_sb_h")
            nc.vector.tensor_scalar_mul(w_sb_h[:], wsum[:], wsc_sb[0:WSH])
            w_int = dpool.tile([WSH, D], f16, tag="w_int", name="w_int")
            nc.sync.dma_start(w_int[:], w_sb_h[:])
            w_all = dpool.tile([3 * D, D], f16, tag="w_all", name="w_all",
                               addr_space="Shared")
            nc.gpsimd.collective_compute(
                "AllGather", mybir.AluOpType.bypass, replica_groups=RG,
                ins=[w_int.opt()], outs=[w_all.opt()])

            def wtiles(base, nm):
                out = []
                for h in range(2):
                    th = cpool.tile([128, D], f16, tag=f"{nm}h{h}",
                                    name=f"{nm}h{h}")
                    nc.sync.dma_start(
                        th[:], w_all[base + h * 128: base + (h + 1) * 128, :])
                    tf = cpool.tile([128, D], f32r, tag=f"{nm}{h}",
                                    name=f"{nm}{h}")
                    nc.vector.tensor_copy(tf[:], th[:])
                    out.append(tf)
                return out

            wq_t = wtiles(0, "wq")
            wkt_t = wtiles(D, "wkt")
            wv_t = wtiles(2 * D, "wv")

            # ---- constants ----
            ones_col = cpool.tile([128, 1], f32r, tag="ones_col", name="ones_col")
            ones_row = cpool.tile([1, 128], f32r, tag="ones_row", name="ones_row")
            ones_f32 = cpool.tile([128, 1], f32, tag="ones_f32", name="ones_f32")
            ones_f32r = cpool.tile([1, 128], f32, tag="ones_f32r", name="ones_f32r")
            bias_t = cpool.tile([128, 1], f32, tag="bias_t", name="bias_t")
            nc.vector.memset(ones_f32[:], 1.0)
            nc.vector.memset(ones_f32r[:], 1.0)
            nc.vector.tensor_copy(ones_col[:], ones_f32[:])
            nc.vector.tensor_copy(ones_row[:], ones_f32r[:])
            nc.vector.memset(bias_t[:], EXP_SHIFT)

            # ---- qT = WQ.T @ xT_local ; M = WK @ qT ----
            qT_t = [ppool.tile([128, NL], f32r, tag=f"qt{h}", name=f"qt{h}") for h in range(2)]
            m_t = [ppool.tile([128, NL], f32r, tag=f"m{h}", name=f"m{h}") for h in range(2)]
            for dst, lhs in ((qT_t, wq_t), (m_t, wkt_t)):
                src = xsT_sb if dst is qT_t else qT_t
                for mh in range(2):
                    for nh in range(2):
                        pp = ps_s.tile([128, 512], f32, tag="scores", name="scores")
                        for kp in range(2):
                            nc.tensor.matmul(
                                pp[:],
                                lhs[kp][:, mh * 128:(mh + 1) * 128],
                                src[kp][:, nh * 512:(nh + 1) * 512],
                                start=(kp == 0), stop=(kp == 1),
                            )
                        nc.vector.tensor_copy(
                            dst[mh][:, nh * 512:(nh + 1) * 512], pp[:])

            # ---- persistent accumulators ----
            ut_ps = [ps_ut.tile([128, NL], f32, tag=f"ut{h}", name=f"ut{h}") for h in range(2)]
            sums_ps = [ps_sum.tile([1, 512], f32, tag=f"sums{h}", name=f"sums{h}")
                       for h in range(2)]

            # ---- main k-loop ----
            for sb in range(N // (KC * SB)):
                xt_t = [xtpool.tile([128, KC * SB], f32r, tag=f"xt{h}", name=f"xt{h}")
                        for h in range(2)]
                for h in range(2):
                    nc.sync.dma_start(
                        xt_t[h][:],
                        xgT[sb * 2 * 128 + h * 128:sb * 2 * 128 + (h + 1) * 128,
                            :])
                xn_t = xnpool.tile([128, SB, D], f32r, tag="xn", name="xn")
                nc.sync.dma_start(
                    xn_t[:],
                    xg[sb * KC * SB:(sb + 1) * KC * SB, :]
                    .rearrange("(a p) d -> p a d", p=128))

                for j in range(SB):
                    c = sb * SB + j
                    first, last = (c == 0), (c == NCHUNK - 1)
                    exps = []
                    for qh in range(2):
                        sp = ps_s.tile([128, 512], f32, tag="scores", name="scores")
                        for kp in range(2):
                            nc.tensor.matmul(
                                sp[:],
                                xt_t[kp][:, j * KC:(j + 1) * KC],
                                m_t[kp][:, qh * 512:(qh + 1) * 512],
                                start=(kp == 0), stop=(kp == 1),
                            )
                        et = epool.tile([128, 512], f32r, tag="expt", name="expt")
                        nc.scalar.activation(et[:], sp[:], EXP, bias=bias_t[:])
                        exps.append(et)
                    for qh in range(2):
                        et = exps[qh]
                        nc.tensor.matmul(
                            sums_ps[qh][:], ones_col[:], et[:],
                            start=first, stop=last)
                        for dh in range(2):
                            nc.tensor.matmul(
                                ut_ps[dh][:, qh * 512:(qh + 1) * 512],
                                xn_t[:, j, dh * 128:(dh + 1) * 128],
                                et[:],
                                start=first, stop=last)

            # ---- tail: softmax normalize + WV projection ----
            sums_sb = tpool.tile([1, NL], f32, tag="sums_sb", name="sums_sb")
            for qh in range(2):
                nc.vector.tensor_copy(
                    sums_sb[:, qh * 512:(qh + 1) * 512], sums_ps[qh][:])
            recip_sb = tpool.tile([1, NL], f32r, tag="recip_sb", name="recip_sb")
            with nc.allow_low_precision(reason="f32r is 4-byte, same mantissa path"):
                nc.vector.reciprocal(recip_sb[:], sums_sb[:])

            rb_sb = tpool.tile([128, NL], f32, tag="rb_sb", name="rb_sb")
            for qh in range(2):
                rp = ps_s.tile([128, 512], f32, tag="scores", name="scores")
                nc.tensor.matmul(
                    rp[:], ones_row[:],
                    recip_sb[:, qh * 512:(qh + 1) * 512],
                    start=True, stop=True)
                nc.vector.tensor_copy(rb_sb[:, qh * 512:(qh + 1) * 512], rp[:])

            utn_sb = [tpool.tile([128, NL], f32r, tag=f"utn{h}", name=f"utn{h}")
                      for h in range(2)]
            for dh in range(2):
                nc.vector.tensor_mul(utn_sb[dh][:], ut_ps[dh][:], rb_sb[:])

            # WV projection into two live PSUM tiles, then int8-quantize with a
            # per-core absmax scale (int8 + f32 scale halves the output bytes;
            # quantization error ~m/240 is ~4e-3 of the rel-err denominator)
            o_f32 = []
            am = tpool.tile([128, 2], f32, tag="am", name="am")
            for mh in range(2):
                op = ps_ut.tile([128, NL], f32, tag=f"ut{mh}", name=f"ut{mh}")
                for nh in range(2):
                    for kp in range(2):
                        nc.tensor.matmul(
                            op[:, nh * 512:(nh + 1) * 512],
                            wv_t[kp][:, mh * 128:(mh + 1) * 128],
                            utn_sb[kp][:, nh * 512:(nh + 1) * 512],
                            start=(kp == 0), stop=(kp == 1),
                        )
                of = tpool.tile([128, NL], f32, tag=f"of{mh}", name=f"of{mh}")
                nc.vector.tensor_copy(of[:], op[:])
                nc.vector.reduce_max(
                    am[:, mh:mh + 1], of[:], axis=mybir.AxisListType.X,
                    apply_absolute_value=True)
                o_f32.append(of)
            amax = tpool.tile([128, 1], f32, tag="amax", name="amax")
            nc.vector.reduce_max(amax[:], am[:], axis=mybir.AxisListType.X)
            nc.gpsimd.partition_all_reduce(
                amax[:], amax[:], channels=128,
                reduce_op=bass_isa.ReduceOp.absmax)
            sc126 = tpool.tile([128, 1], f32, tag="sc126", name="sc126")
            nc.scalar.activation(sc126[:], amax[:], COPY, scale=1.0 / QDEN)
            rcp = tpool.tile([128, 1], f32, tag="rcp", name="rcp")
            nc.vector.reciprocal(rcp[:], sc126[:])
            o_sb = [tpool.tile([128, NL], i8, tag=f"osb{h}", name=f"osb{h}") for h in range(2)]
            for mh in range(2):
                with nc.allow_low_precision(reason="int8 output transport"):
                    nc.vector.tensor_scalar_mul(o_sb[mh][:], o_f32[mh][:],
                                                rcp[:])
                # gpsimd queue, NOT sync: late sync-queue stores corrupt the
                # payload in this build (32-bit words get an fp32-style
                # low-12-bit rounding); the gpsimd DGE ring is clean.
                nc.gpsimd.dma_start(
                    outT[mh * 128:(mh + 1) * 128, :], o_sb[mh][:])
            # absmax f32 bitcast to 4 bytes, packed into outT's extra row
            nc.gpsimd.dma_start(outT[D:D + 1, 0:4],
                                amax[0:1, 0:1].bitcast(i8))

    nc.compile()
    return nc


def _setup_jax_cache():
    """Persistent XLA compilation cache so a fresh process reuses the
    compiled NEFF-wrapped executable instead of re-lowering."""
    if "jaxcache" in _CACHE:
        return
    import jax

    jax.config.update("jax_compilation_cache_dir", "/tmp/jaxcache")
    jax.config.update("jax_persistent_cache_min_entry_size_bytes", 0)
    jax.config.update("jax_persistent_cache_min_compile_time_secs", 0)
    _CACHE["jaxcache"] = True


def _get_nc():
    if "nc" not in _CACHE:
        _setup_jax_cache()
        _CACHE["nc"] = _build()
    return _CACHE["nc"]


def _get_runner():
    """Cached jit of the bass_exec custom call over 8 cores + persistent
    device-resident dummy for the outT operand (never read by the NEFF;
    see module docstring). Mirrors run_bass_kernel_spmd's axon path."""
    if "runner" in _CACHE:
        return _CACHE["runner"]
    import jax
    import concourse.mybir as mybir
    from concourse.bass2jax import _bass_exec_p, install_neuronx_cc_hook
    from jax.experimental.shard_map import shard_map
    from jax.sharding import Mesh, NamedSharding, PartitionSpec

    nc = _get_nc()
    install_neuronx_cc_hook()

    in_names, out_names, out_avals = [], [], []
    for alloc in nc.m.functions[0].allocations:
        if not isinstance(alloc, mybir.MemoryLocationSet):
            continue
        name = alloc.memorylocations[0].name
        if alloc.kind == "ExternalInput":
            in_names.append(name)
        elif alloc.kind == "ExternalOutput":
            out_names.append(name)
            out_avals.append(jax.core.ShapedArray(
                tuple(alloc.tensor_shape), mybir.dt.np(alloc.dtype)))
    in_names = in_names + out_names

    def _body(*args):
        return tuple(_bass_exec_p.bind(
            *args,
            out_avals=tuple(out_avals),
            in_names=tuple(in_names),
            out_names=tuple(out_names),
            lowering_input_output_aliases=(),
            sim_require_finite=True,
            sim_require_nnan=True,
            nc=nc,
        ))

    devices = jax.devices()[:P]
    mesh = Mesh(np.asarray(devices), ("core",))
    fn = jax.jit(shard_map(_body, mesh=mesh,
                           in_specs=(PartitionSpec("core"),) * len(in_names),
                           out_specs=(PartitionSpec("core"),) * len(out_names),
                           check_rep=False),
                 keep_unused=True)
    oav = out_avals[0]
    dummy = jax.device_put(
        np.zeros((P * oav.shape[0], *oav.shape[1:]), oav.dtype),
        NamedSharding(mesh, PartitionSpec("core")))
    dummy.block_until_ready()
    _CACHE["runner"] = (fn, dummy)
    return _CACHE["runner"]


def _pack12(a):
    """Per-row 12-bit fixed point: returns (scales f32 [n], hi int8 [n,d],
    lopack u8 [n,d/2]) with lopack[:,c] = lo[:,c] | lo[:,c+d/2]<<4."""
    h = a.shape[1] // 2
    s = np.abs(a).max(axis=1)
    s /= 2047.0
    y = a * (1.0 / s)[:, None]
    np.rint(y, out=y)
    v12 = y.astype(np.int16)
    hi = (v12 >> 4).astype(np.int8)
    np.bitwise_and(v12, 15, out=v12)
    lo = v12.astype(np.uint8)
    lopack = lo[:, h:]
    np.left_shift(lopack, 4, out=lopack)
    np.bitwise_or(lopack, lo[:, :h], out=lopack)
    return s, hi, lopack


def make_blob(input, WQ, WK, WV):
    """Global [8*1698, 256] uint8 input blob (all cores concatenated)."""
    x = np.ascontiguousarray(input, dtype=np.float32)
    s, hi, lopack = _pack12(x)
    wpack = np.concatenate(
        [np.asarray(WQ, dtype=np.float32),
         np.asarray(WK, dtype=np.float32).T,
         np.asarray(WV, dtype=np.float32)], axis=0)
    ws, whi, wlopack = _pack12(wpack)
    wsc_pad = np.zeros((P, 128), np.float32)
    wsc_pad[:, :WSH] = ws.reshape(P, WSH)

    blob = np.empty((P * RB, D), np.uint8)
    for c in range(P):
        b = blob[c * RB:(c + 1) * RB]
        r0, r1 = c * NL, (c + 1) * NL
        b[R_HI0:R_HI1] = hi[r0:r1].view(np.uint8)
        b[R_LO0:R_LO1] = lopack[r0:r1].reshape(512, 256)
        b[R_SC0:R_SC1] = np.ascontiguousarray(
            s[r0:r1].reshape(SB, 128).T).view(np.uint8).reshape(16, 256)
        w0, w1 = c * WSH, (c + 1) * WSH
        b[R_WH0:R_WH1] = whi[w0:w1].view(np.uint8)
        b[R_WL0:R_WL1] = wlopack[w0:w1].reshape(48, 256)
        b[R_WS0:R_WS1] = wsc_pad[c].view(np.uint8).reshape(2, 256)
    return blob


def _execute(blob):
    """The timed hot path: one jit call (h2d blob, NEFF exec, d2h outT)."""
    fn, dummy = _get_runner()
    out = fn(blob, dummy)
    return np.asarray(out[0])


def _unshard(o_all):
    res = o_all.reshape(P, D + 1, NL)
    out = np.empty((N, D), dtype=np.float32)
    for c in range(P):
        o = res[c]
        amax = np.frombuffer(o[D, 0:4].tobytes(), np.float32)[0]
        np.multiply(o[:D].T, np.float32(amax / QDEN),
                    out=out[c * NL:(c + 1) * NL, :], casting="unsafe")
    return out


def kernel(input, WQ, WK, WV):
    _get_runner()
    blob = make_blob(input, WQ, WK, WV)
    return _unshard(_execute(blob))
